# revision 1
# baseline (speedup 1.0000x reference)
"""Grouped-Query Attention kernel for Trainium2 (8 NeuronCores, SPMD).

Problem: x [4, 4096, 512] fp32, per-group Dense Q/K/V (G=4 groups of 128
features), full softmax attention within each (batch, group) pair, output
re-concatenated to [4, 4096, 512].

Sharding: B*G = 16 fully independent attention problems -> 2 per core.
Per core, per pair, everything stays on-chip (SBUF 24MB):
  - load xg [4096, 128] fp32, cast bf16, PE-transpose to xgT [d, t]
  - Q^T = Wq^T xg^T, K^T likewise (bias folded in), V natural [t, e]
  - scores computed TRANSPOSED: S^T[ts, tq] = K Q^T so that the exp'd
    probabilities land directly in the layout attn@V needs as rhs
    (contraction dim ts on partitions) -- no transpose of the TxT matrix.
  - exp via ScalarE with the 1/sqrt(gs) scale folded into ACT's free affine
  - softmax denominator via an extra ones-matmul pass (all-rows-equal
    accumulate), out^T accumulated over ts chunks in PSUM
  - epilogue: reciprocal, normalize, +bv, PE-transpose back to natural
Compute dtype bf16 (fp32 accumulation in PSUM).
"""

import os
import sys

sys.path.insert(0, "/opt/trn_rl_repo")

import numpy as np

import concourse.bass as bass
import concourse.mybir as mybir
import concourse.tile as tile
from concourse.masks import make_identity

B, T, F, G = 4, 4096, 512, 4
GS = F // G  # 128
N_CORES = 8
PAIRS_PER_CORE = (B * G) // N_CORES  # 2
TQ_MACRO = 1024  # query tile width per softmax/psum round
N_MACROS = T // TQ_MACRO  # 4
N_CHUNKS = T // 128  # 32 key/time chunks
INV_SCALE = float(1.0 / (np.sqrt(np.float32(GS)) + 1e-9))

FP32 = mybir.dt.float32
BF16 = mybir.dt.bfloat16

_NC_CACHE = None
_LAST_IN_MAPS = None


def _split_multi_waits(nc):
    """Walrus codegen rejects instructions carrying more than one semaphore
    wait on several instruction structs (DMA DIRECT2D, tensor_scalar, LDW).
    Hoist all-but-the-last wait of any multi-wait instruction onto same-engine
    NoOps inserted immediately before it: the sequencer executes them in
    order, so the gating semantics are identical."""
    n_split = 0
    for func in nc.m.functions:
        for block in func.blocks:
            new = []
            for inst in block.instructions:
                si = inst.sync_info
                waits = list(si.on_wait) if (si is not None and si.on_wait) else []
                if len(waits) > 1:
                    for w in waits[:-1]:
                        nop = mybir.InstNoOp(
                            name=nc.get_next_instruction_name(), ins=[], outs=[]
                        )
                        nop.engine = inst.engine
                        nop.sync_info = mybir.SyncInfo(on_wait=[w], on_update=[])
                        new.append(nop)
                        n_split += 1
                    inst.sync_info = mybir.SyncInfo(
                        on_wait=[waits[-1]],
                        on_update=list(si.on_update) if si.on_update else [],
                    )
                new.append(inst)
            block.instructions = new
    return n_split


def build_nc():
    nc = bass.Bass()

    ins = []
    outs = []
    for i in range(PAIRS_PER_CORE):
        ins.append(
            dict(
                x=nc.declare_dram_parameter(f"x{i}", [T, GS], FP32, isOutput=False),
                wq=nc.declare_dram_parameter(f"wq{i}", [GS, GS], FP32, isOutput=False),
                wk=nc.declare_dram_parameter(f"wk{i}", [GS, GS], FP32, isOutput=False),
                wv=nc.declare_dram_parameter(f"wv{i}", [GS, GS], FP32, isOutput=False),
                bq=nc.declare_dram_parameter(f"bq{i}", [1, GS], FP32, isOutput=False),
                bk=nc.declare_dram_parameter(f"bk{i}", [1, GS], FP32, isOutput=False),
                bv=nc.declare_dram_parameter(f"bv{i}", [1, GS], FP32, isOutput=False),
            )
        )
        outs.append(nc.declare_dram_parameter(f"y{i}", [T, GS], FP32, isOutput=True))

    with tile.TileContext(nc) as tc:
        with (
            tc.tile_pool(name="consts", bufs=1) as consts,
            tc.tile_pool(name="bigsb", bufs=2) as bigsb,  # per-pair persistent
            tc.tile_pool(name="pt", bufs=4) as ptpool,  # exp'd prob chunks
            tc.tile_pool(name="epi", bufs=2) as epi,  # epilogue sbuf tiles
            tc.tile_pool(name="ps_s", bufs=2, space="PSUM") as ps_s,  # scores
            tc.tile_pool(name="ps_o", bufs=1, space="PSUM") as ps_o,  # out^T
            tc.tile_pool(name="ps_d", bufs=1, space="PSUM") as ps_d,  # denom
        ):
            ident_bf = consts.tile([128, 128], BF16)
            make_identity(nc, ident_bf)
            ident_f = consts.tile([128, 128], FP32)
            make_identity(nc, ident_f)
            ones_bf = consts.tile([128, 128], BF16)
            nc.vector.memset(ones_bf, 1.0)

            for i in range(PAIRS_PER_CORE):
                p = ins[i]
                # ---------------- prologue: load + QKV ----------------
                xg_f = bigsb.tile([128, N_CHUNKS, 128], FP32, tag="xg_f")
                nc.sync.dma_start(
                    out=xg_f, in_=p["x"][:, :].rearrange("(c p) d -> p c d", p=128)
                )
                xg_b = bigsb.tile([128, N_CHUNKS, 128], BF16, tag="xg_b")
                nc.vector.tensor_copy(xg_b, xg_f)

                # weights + biases
                w_bf = {}
                for nm in ("wq", "wk", "wv"):
                    wf = epi.tile([128, 128], FP32, tag=f"wf{nm}{i}")
                    nc.gpsimd.dma_start(out=wf, in_=p[nm][:, :])
                    wb = consts.tile([128, 128], BF16, tag=f"{nm}{i}")
                    nc.vector.tensor_copy(wb, wf)
                    w_bf[nm] = wb
                b_col = {}
                for nm in ("bq", "bk", "bv"):
                    bc = consts.tile([128, 1], FP32, tag=f"{nm}{i}")
                    nc.gpsimd.dma_start(
                        out=bc, in_=p[nm][:, :].rearrange("o d -> d o")
                    )
                    b_col[nm] = bc
                bvb = consts.tile([128, 128], FP32, tag=f"bvb{i}")
                _bv = p["bv"][:, :]
                nc.gpsimd.dma_start(
                    out=bvb,
                    in_=bass.AP(tensor=_bv.tensor, offset=_bv.offset,
                                ap=[[0, 128]] + list(_bv.ap[1:])),
                )

                # xgT [d, t] bf16 via PE transpose of 32 chunks
                xgT = bigsb.tile([128, T], BF16, tag="xgT")
                for c in range(N_CHUNKS):
                    pst = ps_s.tile([128, 128], BF16, tag="sc")
                    nc.tensor.transpose(pst, xg_b[:, c, :], ident_bf)
                    nc.vector.tensor_copy(xgT[:, c * 128 : (c + 1) * 128], pst)

                # Q^T/K^T [e, t] bf16 (bias added), V^T -> V natural
                qt = bigsb.tile([128, T], BF16, tag="qt")
                kt = bigsb.tile([128, T], BF16, tag="kt")
                vt = bigsb.tile([128, T], BF16, tag="vt")
                for dst, wname, bname in (
                    (qt, "wq", "bq"),
                    (kt, "wk", "bk"),
                    (vt, "wv", None),
                ):
                    for j in range(T // TQ_MACRO):
                        psq = ps_s.tile([128, TQ_MACRO], FP32, tag="sc")
                        for h in range(TQ_MACRO // 512):
                            sl = slice(h * 512, (h + 1) * 512)
                            tsl = slice(j * TQ_MACRO + h * 512, j * TQ_MACRO + (h + 1) * 512)
                            nc.tensor.matmul(
                                psq[:, sl], w_bf[wname], xgT[:, tsl], start=True, stop=True
                            )
                        dsl = slice(j * TQ_MACRO, (j + 1) * TQ_MACRO)
                        if bname is not None:
                            nc.vector.tensor_scalar_add(dst[:, dsl], psq, b_col[bname])
                        else:
                            nc.vector.tensor_copy(dst[:, dsl], psq)

                v_nat = bigsb.tile([128, N_CHUNKS, 128], BF16, tag="v_nat")
                for c in range(N_CHUNKS):
                    pst = ps_s.tile([128, 128], BF16, tag="sc")
                    nc.tensor.transpose(pst, vt[:, c * 128 : (c + 1) * 128], ident_bf)
                    nc.vector.tensor_copy(v_nat[:, c, :], pst)

                # ---------------- attention macros ----------------
                for m in range(N_MACROS):
                    tq0 = m * TQ_MACRO
                    ps_out = ps_o.tile([128, TQ_MACRO], FP32)
                    ps_den = ps_d.tile([128, TQ_MACRO], FP32)
                    for c in range(N_CHUNKS):
                        ksl = kt[:, c * 128 : (c + 1) * 128]
                        ps_sc = ps_s.tile([128, TQ_MACRO], FP32, tag="sc")
                        for h in range(TQ_MACRO // 512):
                            sl = slice(h * 512, (h + 1) * 512)
                            qsl = slice(tq0 + h * 512, tq0 + (h + 1) * 512)
                            nc.tensor.matmul(
                                ps_sc[:, sl], ksl, qt[:, qsl], start=True, stop=True
                            )
                        pt = ptpool.tile([128, TQ_MACRO], BF16)
                        nc.scalar.activation(
                            pt, ps_sc, mybir.ActivationFunctionType.Exp, scale=INV_SCALE
                        )
                        first, last = c == 0, c == N_CHUNKS - 1
                        for h in range(TQ_MACRO // 512):
                            sl = slice(h * 512, (h + 1) * 512)
                            nc.tensor.matmul(
                                ps_out[:, sl], v_nat[:, c, :], pt[:, sl],
                                start=first, stop=last,
                            )
                            nc.tensor.matmul(
                                ps_den[:, sl], ones_bf, pt[:, sl],
                                start=first, stop=last,
                            )
                    recip = epi.tile([128, TQ_MACRO], FP32, tag="recip")
                    nc.vector.reciprocal(recip, ps_den)
                    onorm = epi.tile([128, TQ_MACRO], FP32, tag="onorm")
                    nc.vector.tensor_mul(onorm, ps_out, recip)
                    nc.vector.tensor_scalar_add(onorm, onorm, b_col["bv"])
                    onat = epi.tile([128, TQ_MACRO // 128, 128], FP32, tag="onat")
                    for j in range(TQ_MACRO // 128):
                        pst = ps_s.tile([128, 128], FP32, tag="sc")
                        nc.tensor.transpose(pst, onorm[:, j * 128 : (j + 1) * 128], ident_f)
                        nc.vector.tensor_copy(onat[:, j, :], pst)
                    nc.sync.dma_start(
                        out=outs[i][tq0 : tq0 + TQ_MACRO, :].rearrange(
                            "(c p) d -> p c d", p=128
                        ),
                        in_=onat,
                    )
    _split_multi_waits(nc)
    return nc


def _get_nc():
    global _NC_CACHE
    if _NC_CACHE is None:
        _NC_CACHE = build_nc()
    return _NC_CACHE


def kernel(**inputs: np.ndarray) -> np.ndarray:
    x = np.ascontiguousarray(inputs["x"], dtype=np.float32)
    Wq = np.asarray(inputs["Wq"], dtype=np.float32)
    Wk = np.asarray(inputs["Wk"], dtype=np.float32)
    Wv = np.asarray(inputs["Wv"], dtype=np.float32)
    bq = np.asarray(inputs["bq"], dtype=np.float32)
    bk = np.asarray(inputs["bk"], dtype=np.float32)
    bv = np.asarray(inputs["bv"], dtype=np.float32)

    nc = _get_nc()

    in_maps = []
    for core in range(N_CORES):
        m = {}
        for i in range(PAIRS_PER_CORE):
            pair = core * PAIRS_PER_CORE + i
            b, g = pair // G, pair % G
            sl = slice(g * GS, (g + 1) * GS)
            m[f"x{i}"] = np.ascontiguousarray(x[b, :, sl])
            m[f"wq{i}"] = np.ascontiguousarray(Wq[g])
            m[f"wk{i}"] = np.ascontiguousarray(Wk[g])
            m[f"wv{i}"] = np.ascontiguousarray(Wv[g])
            m[f"bq{i}"] = np.ascontiguousarray(bq[g].reshape(1, GS))
            m[f"bk{i}"] = np.ascontiguousarray(bk[g].reshape(1, GS))
            m[f"bv{i}"] = np.ascontiguousarray(bv[g].reshape(1, GS))
        in_maps.append(m)

    global _LAST_IN_MAPS
    _LAST_IN_MAPS = in_maps

    from concourse.bass_utils import run_bass_kernel_spmd

    res = run_bass_kernel_spmd(nc, in_maps, list(range(N_CORES)))

    y = np.empty((B, T, F), dtype=np.float32)
    for core in range(N_CORES):
        for i in range(PAIRS_PER_CORE):
            pair = core * PAIRS_PER_CORE + i
            b, g = pair // G, pair % G
            y[b, :, g * GS : (g + 1) * GS] = res.results[core][f"y{i}"]
    return y



# revision 18
# speedup vs baseline: 1.7992x; 1.7992x over previous
"""Grouped-Query Attention kernel for Trainium2 (8 NeuronCores, SPMD).

Problem: x [4, 4096, 512] fp32, per-group Dense Q/K/V (G=4 groups of 128
features), full softmax attention within each (batch, group) pair, output
re-concatenated to [4, 4096, 512].

Sharding: B*G = 16 fully independent attention problems -> 2 per core.

v2 design (software-pipelined, fp16 compute):
  - prologue: load xg, cast fp16, PE-transpose to xgT [d, t];
    Q^T/K^T = W^T xgT (bias folded); V computed NATURAL directly via
    matmul(lhsT=xgT_chunk, rhs=Wv) with bv folded in (exact: softmax
    weights sum to 1, so out = P(V+bv)/den == PV/den + bv).
  - scores transposed: S^T[ts, tq] = K^T(lhsT) @ Q^T chunks; exp via ACT
    with 1/sqrt(gs) folded into the scale; probabilities pt land in fp16.
  - denominator: pt chunk pairs summed on DVE (fp16 2x mode), then 8
    one-column matmuls (lhsT=pair slice, rhs=ones) accumulate den^T[tq,1]
    directly in natural orientation -> tiny [128,8] PSUM tile per macro.
  - out^T accumulated over ts chunks in PSUM; epilogue per macro: DVE
    drains ps_out -> SBUF, approx-fast reciprocal of [128,8] den,
    PE-transpose 128-blocks to natural, per-partition tensor_scalar_mul
    by recip, DMA out.
  - emission is software-pipelined: out-matmuls lag 2 chunks behind
    scores/exp; the previous macro's epilogue is spread over slots in the
    first chunks of the next macro. PE never han an intentional idle slot
    (TRN2 PE drops to half clock after any idle gap).
"""

import os
import sys

sys.path.insert(0, "/opt/trn_rl_repo")

import numpy as np

import concourse.bass as bass
import concourse.mybir as mybir
import concourse.tile as tile
from concourse.masks import make_identity

B, T, F, G = 4, 4096, 512, 4
GS = F // G  # 128
N_CORES = 8
PAIRS_PER_CORE = (B * G) // N_CORES  # 2
TQ = 1024  # query tile width per macro
NM = T // TQ  # 4 macros
NCH = T // 128  # 32 key/time chunks
INV_SCALE = float(1.0 / (np.sqrt(np.float32(GS)) + 1e-9))

FP32 = mybir.dt.float32
FP16 = mybir.dt.float16

_NC_CACHE = None
_LAST_IN_MAPS = None


def _split_multi_waits(nc):
    """Walrus codegen rejects instructions carrying more than one semaphore
    wait on several instruction structs (DMA DIRECT2D, tensor_scalar, LDW).
    Hoist all-but-the-last wait of any multi-wait instruction onto same-engine
    NoOps inserted immediately before it: the sequencer executes them in
    order, so the gating semantics are identical."""
    n_split = 0
    for func in nc.m.functions:
        for block in func.blocks:
            new = []
            for inst in block.instructions:
                si = inst.sync_info
                waits = list(si.on_wait) if (si is not None and si.on_wait) else []
                if len(waits) > 1:
                    for w in waits[:-1]:
                        nop = mybir.InstNoOp(
                            name=nc.get_next_instruction_name(), ins=[], outs=[]
                        )
                        nop.engine = inst.engine
                        nop.sync_info = mybir.SyncInfo(on_wait=[w], on_update=[])
                        new.append(nop)
                        n_split += 1
                    inst.sync_info = mybir.SyncInfo(
                        on_wait=[waits[-1]],
                        on_update=list(si.on_update) if si.on_update else [],
                    )
                new.append(inst)
            block.instructions = new
    return n_split


def build_nc():
    nc = bass.Bass()

    ins = []
    outs = []
    outs_dbg = None
    if os.environ.get("DBG_DEN") == "1":
        outs_dbg = nc.declare_dram_parameter("dbg0", [128, 16], FP32, isOutput=True)
    for i in range(PAIRS_PER_CORE):
        ins.append(
            dict(
                x=nc.declare_dram_parameter(f"x{i}", [T, GS], FP32, isOutput=False),
                wq=nc.declare_dram_parameter(f"wq{i}", [GS, GS], FP32, isOutput=False),
                wk=nc.declare_dram_parameter(f"wk{i}", [GS, GS], FP32, isOutput=False),
                wv=nc.declare_dram_parameter(f"wv{i}", [GS, GS], FP32, isOutput=False),
                bq=nc.declare_dram_parameter(f"bq{i}", [1, GS], FP32, isOutput=False),
                bk=nc.declare_dram_parameter(f"bk{i}", [1, GS], FP32, isOutput=False),
                bv=nc.declare_dram_parameter(f"bv{i}", [1, GS], FP32, isOutput=False),
            )
        )
        outs.append(nc.declare_dram_parameter(f"y{i}", [T, GS], FP32, isOutput=True))

    with tile.TileContext(nc) as tc:
        with (
            tc.tile_pool(name="consts", bufs=1) as consts,
            tc.tile_pool(name="bigsb", bufs=2) as bigsb,  # per-pair persistent
            tc.tile_pool(name="ptp", bufs=4) as ptp,  # exp'd prob chunks
            tc.tile_pool(name="ppair", bufs=2) as ppair,  # chunk-pair sums
            tc.tile_pool(name="epi", bufs=2) as epi,  # epilogue sbuf tiles
            tc.tile_pool(name="ps", bufs=2, space="PSUM") as ps,  # scores + misc
            tc.tile_pool(name="ps_o", bufs=1, space="PSUM") as ps_o,  # out^T
            tc.tile_pool(name="ps_dn", bufs=1, space="PSUM") as ps_dn,  # den^T
            tc.tile_pool(name="ps_ep", bufs=1, space="PSUM") as ps_ep,  # nat out
        ):
            ident_h = consts.tile([128, 128], FP16)
            make_identity(nc, ident_h)
            ident_f = consts.tile([128, 128], FP32)
            make_identity(nc, ident_f)
            ones_col = consts.tile([128, 1], FP16)
            nc.vector.memset(ones_col, 1.0)

            # ------------- input DMAs for both pairs up front -------------
            pair_in = []
            for i in range(PAIRS_PER_CORE):
                p = ins[i]
                xg_f = bigsb.tile([128, NCH, 128], FP32, tag="xg_f")
                nc.sync.dma_start(
                    out=xg_f, in_=p["x"][:, :].rearrange("(c p) d -> p c d", p=128)
                )
                w_f = {}
                for nm in ("wq", "wk", "wv"):
                    wf = consts.tile([128, 128], FP32, tag=f"wf_{nm}{i}")
                    nc.gpsimd.dma_start(out=wf, in_=p[nm][:, :])
                    w_f[nm] = wf
                b_col = {}
                for nm in ("bq", "bk"):
                    bc = consts.tile([128, 1], FP32, tag=f"{nm}{i}")
                    nc.gpsimd.dma_start(
                        out=bc, in_=p[nm][:, :].rearrange("o d -> d o")
                    )
                    b_col[nm] = bc
                bvb = consts.tile([128, 128], FP32, tag=f"bvb{i}")
                _bv = p["bv"][:, :]
                nc.gpsimd.dma_start(
                    out=bvb,
                    in_=bass.AP(tensor=_bv.tensor, offset=_bv.offset,
                                ap=[[0, 128]] + list(_bv.ap[1:])),
                )
                pair_in.append(dict(xg_f=xg_f, w_f=w_f, b_col=b_col, bvb=bvb))

            # ------------- prologues (both pairs) -------------
            pair_data = []
            for i in range(PAIRS_PER_CORE):
                pi = pair_in[i]
                w_h = {}
                for nm in ("wq", "wk", "wv"):
                    wh = consts.tile([128, 128], FP16, tag=f"wh_{nm}{i}")
                    nc.vector.tensor_copy(wh, pi["w_f"][nm])
                    w_h[nm] = wh

                xg_h = bigsb.tile([128, NCH, 128], FP16, tag="xg_h")
                xgT = bigsb.tile([128, T], FP16, tag="xgT")
                # cast in 4 slabs; transpose each slab's 8 chunks right after
                for k in range(4):
                    nc.vector.tensor_copy(
                        xg_h[:, k * 8 : (k + 1) * 8, :], pi["xg_f"][:, k * 8 : (k + 1) * 8, :]
                    )
                    for c in range(k * 8, (k + 1) * 8):
                        pst = ps.tile([128, 128], FP16, tag="sc")
                        nc.tensor.transpose(pst, xg_h[:, c, :], ident_h)
                        if c % 2 == 0:
                            nc.vector.tensor_copy(xgT[:, c * 128 : (c + 1) * 128], pst)
                        else:
                            nc.scalar.copy(xgT[:, c * 128 : (c + 1) * 128], pst)

                # Q^T, K^T with bias
                qt = bigsb.tile([128, T], FP16, tag="qt")
                kt = bigsb.tile([128, T], FP16, tag="kt")
                for dst, wname, bname in ((qt, "wq", "bq"), (kt, "wk", "bk")):
                    for j in range(NM):
                        psq = ps.tile([128, TQ], FP32, tag="sc")
                        for h in range(2):
                            sl = slice(h * 512, (h + 1) * 512)
                            tsl = slice(j * TQ + h * 512, j * TQ + (h + 1) * 512)
                            nc.tensor.matmul(
                                psq[:, sl], w_h[wname], xgT[:, tsl],
                                start=True, stop=True,
                            )
                        dsl = slice(j * TQ, (j + 1) * TQ)
                        nc.scalar.add(dst[:, dsl], psq, pi["b_col"][bname])

                # V natural directly: v_nat[ts, e] = xgT_chunk^T @ Wv
                # (bv is folded into the epilogue normalize: out = PV/den + bv)
                v_nat = bigsb.tile([128, NCH, 128], FP16, tag="v_nat")
                for c in range(NCH):
                    psv = ps.tile([128, 128], FP32, tag="sc")
                    nc.tensor.matmul(
                        psv, xgT[:, c * 128 : (c + 1) * 128], w_h["wv"],
                        start=True, stop=True,
                    )
                    if c % 2 == 0:
                        nc.vector.tensor_copy(v_nat[:, c, :], psv)
                    else:
                        nc.scalar.copy(v_nat[:, c, :], psv)

                pair_data.append(dict(qt=qt, kt=kt, v_nat=v_nat, bvb=pi["bvb"]))

            # ------------- attention: software-pipelined macro loop -------
            # prev: state of the previous macro whose epilogue is pending.
            prev = None

            def emit_epilogue_slot(c, st):
                """Emit the slice of the previous macro's epilogue assigned
                to chunk-slot c of the current macro stream."""
                if st is None:
                    return
                if c == 1:
                    # drain ps_out -> SBUF (DVE), reciprocal of den
                    nc.vector.tensor_copy(st["osb"], st["ps_out"])
                    nc.vector.reciprocal_approx_fast(
                        out=st["rcol"], in_=st["ps_den"]
                    )
                    if st.get("dbg") is not None:
                        dbg_sb = consts.tile([128, 16], FP32, tag="dbg_sb")
                        nc.vector.tensor_copy(dbg_sb[:, 0:8], st["ps_den"])
                        nc.vector.tensor_copy(dbg_sb[:, 8:16], st["rcol"])
                        st["dbg_sb"] = dbg_sb
                elif c in (3, 4):
                    j0 = 0 if c == 3 else 4
                    for j in range(j0, j0 + 4):
                        q = j % 4
                        pse = st["ps_ep_t"]
                        nc.tensor.transpose(
                            pse[:, q * 128 : (q + 1) * 128],
                            st["osb"][:, j * 128 : (j + 1) * 128],
                            ident_f,
                        )
                        # out = (outT.T * 1/den) + bv  (bvb: partition-
                        # constant broadcast of bv, varies along e = free)
                        nc.vector.scalar_tensor_tensor(
                            out=st["out_sb"][:, j, :],
                            in0=pse[:, q * 128 : (q + 1) * 128],
                            scalar=st["rcol"][:, j : j + 1],
                            in1=st["bvb"],
                            op0=mybir.AluOpType.mult,
                            op1=mybir.AluOpType.add,
                        )
                elif c == 5:
                    nc.sync.dma_start(
                        out=st["y"][st["tq0"] : st["tq0"] + TQ, :].rearrange(
                            "(c p) d -> p c d", p=128
                        ),
                        in_=st["out_sb"],
                    )


            def emit_flush(st_cur, ptl, ppl):
                """Final out/den matmuls of the current macro (lag drain)."""
                for cc in (NCH - 2, NCH - 1):
                    for h in range(2):
                        sl = slice(h * 512, (h + 1) * 512)
                        nc.tensor.matmul(
                            st_cur["ps_out"][:, sl],
                            st_cur["v_nat"][:, cc, :],
                            ptl[cc][:, sl],
                            start=(cc == 0), stop=(cc == NCH - 1),
                        )
                # NOTE: stop=True only on the final matmul touching the bank.
                # A matmul with start=False, stop=True whose write is NOT the
                # bank-final one loses its accumulation on TRN2 hardware
                # (observed: den columns 0..6 missing the last pair's sum).
                pp = NCH // 2 - 1  # last pair
                for j in range(8):
                    nc.tensor.matmul(
                        st_cur["ps_den"][:, j : j + 1],
                        ppl[pp][:, j * 128 : (j + 1) * 128],
                        ones_col,
                        start=(pp == 0), stop=(j == 7),
                    )

            for i in range(PAIRS_PER_CORE):
                pd = pair_data[i]
                qt, kt, v_nat = pd["qt"], pd["kt"], pd["v_nat"]
                for m in range(NM):
                    tq0 = m * TQ
                    ps_out = ps_o.tile([128, TQ], FP32, tag="o")
                    ps_den = ps_dn.tile([128, 8], FP32, tag="dn")
                    ptl = [None] * NCH
                    ppl = [None] * (NCH // 2)
                    st_cur = dict(
                        ps_out=ps_out, ps_den=ps_den, v_nat=v_nat, tq0=tq0,
                        y=outs[i], bvb=pd["bvb"],
                        dbg=outs_dbg if (i == 1 and m == NM - 1) else None,
                    )
                    for c in range(NCH):
                        # scores for chunk c
                        sc_t = ps.tile([128, TQ], FP32, tag="sc")
                        ksl = kt[:, c * 128 : (c + 1) * 128]
                        for h in range(2):
                            sl = slice(h * 512, (h + 1) * 512)
                            qsl = slice(tq0 + h * 512, tq0 + (h + 1) * 512)
                            nc.tensor.matmul(
                                sc_t[:, sl], ksl, qt[:, qsl], start=True, stop=True
                            )
                        pt_c = ptp.tile([128, TQ], FP16, tag="pt")
                        nc.scalar.activation(
                            pt_c, sc_t, mybir.ActivationFunctionType.Exp,
                            scale=INV_SCALE,
                        )
                        ptl[c] = pt_c

                        # previous macro's epilogue rides in early slots
                        emit_epilogue_slot(c, prev)

                        # lagged out-matmuls (chunk c-2)
                        if c >= 2:
                            cc = c - 2
                            for h in range(2):
                                sl = slice(h * 512, (h + 1) * 512)
                                nc.tensor.matmul(
                                    ps_out[:, sl], v_nat[:, cc, :], ptl[cc][:, sl],
                                    start=(cc == 0), stop=False,
                                )

                        # chunk-pair sum; lagged den matmuls
                        if c % 2 == 1:
                            pp_i = (c - 1) // 2
                            pp_t = ppair.tile([128, TQ], FP16, tag="pp")
                            nc.vector.tensor_add(pp_t, ptl[c - 1], ptl[c])
                            ppl[pp_i] = pp_t
                            if c >= 3:
                                dp = (c - 3) // 2
                                for j in range(8):
                                    # start=True ONLY on the very first
                                    # column-matmul: a start marks the whole
                                    # 2KB psum zero-region pending-zero, so
                                    # start on later columns would discard
                                    # the columns written just before.
                                    nc.tensor.matmul(
                                        ps_den[:, j : j + 1],
                                        ppl[dp][:, j * 128 : (j + 1) * 128],
                                        ones_col,
                                        start=(dp == 0 and j == 0), stop=False,
                                    )
                    # flush the lag of this macro
                    emit_flush(st_cur, ptl, ppl)
                    # hand off epilogue state
                    osb = epi.tile([128, TQ], FP32, tag="osb")
                    rcol = epi.tile([128, 8], FP32, tag="rcol")
                    out_sb = epi.tile([128, TQ // 128, 128], FP32, tag="out_sb")
                    ps_ep_t = ps_ep.tile([128, 512], FP32, tag="ep")
                    st_cur.update(osb=osb, rcol=rcol, out_sb=out_sb, ps_ep_t=ps_ep_t)
                    prev = st_cur

            # tail: last macro's epilogue, emitted standalone
            for c in (1, 3, 4, 5):
                emit_epilogue_slot(c, prev)
            if prev.get("dbg") is not None:
                # late re-read of the final macro's den psum, after everything
                nc.vector.tensor_copy(prev["dbg_sb"][:, 8:16], prev["ps_den"])
                nc.sync.dma_start(out=prev["dbg"][:, :], in_=prev["dbg_sb"])

    # populate .instr bytes for extended InstISA ops (custom DVE reciprocal);
    # raw Bass skips this pass and walrus then fails with "ISA wrong length".
    mybir.codegen_inst_isa_subclasses(nc)
    _split_multi_waits(nc)
    return nc


def _get_nc():
    global _NC_CACHE
    if _NC_CACHE is None:
        _NC_CACHE = build_nc()
    return _NC_CACHE


def kernel(**inputs: np.ndarray) -> np.ndarray:
    x = np.ascontiguousarray(inputs["x"], dtype=np.float32)
    Wq = np.asarray(inputs["Wq"], dtype=np.float32)
    Wk = np.asarray(inputs["Wk"], dtype=np.float32)
    Wv = np.asarray(inputs["Wv"], dtype=np.float32)
    bq = np.asarray(inputs["bq"], dtype=np.float32)
    bk = np.asarray(inputs["bk"], dtype=np.float32)
    bv = np.asarray(inputs["bv"], dtype=np.float32)

    nc = _get_nc()

    in_maps = []
    for core in range(N_CORES):
        m = {}
        for i in range(PAIRS_PER_CORE):
            pair = core * PAIRS_PER_CORE + i
            b, g = pair // G, pair % G
            sl = slice(g * GS, (g + 1) * GS)
            m[f"x{i}"] = np.ascontiguousarray(x[b, :, sl])
            m[f"wq{i}"] = np.ascontiguousarray(Wq[g])
            m[f"wk{i}"] = np.ascontiguousarray(Wk[g])
            m[f"wv{i}"] = np.ascontiguousarray(Wv[g])
            m[f"bq{i}"] = np.ascontiguousarray(bq[g].reshape(1, GS))
            m[f"bk{i}"] = np.ascontiguousarray(bk[g].reshape(1, GS))
            m[f"bv{i}"] = np.ascontiguousarray(bv[g].reshape(1, GS))
        in_maps.append(m)

    global _LAST_IN_MAPS
    _LAST_IN_MAPS = in_maps

    from concourse.bass_utils import run_bass_kernel_spmd

    res = run_bass_kernel_spmd(nc, in_maps, list(range(N_CORES)))

    y = np.empty((B, T, F), dtype=np.float32)
    for core in range(N_CORES):
        for i in range(PAIRS_PER_CORE):
            pair = core * PAIRS_PER_CORE + i
            b, g = pair // G, pair % G
            y[b, :, g * GS : (g + 1) * GS] = res.results[core][f"y{i}"]
    return y


# revision 21
# speedup vs baseline: 1.9411x; 1.0788x over previous
"""Grouped-Query Attention kernel for Trainium2 (8 NeuronCores, SPMD).

Problem: x [4, 4096, 512] fp32, per-group Dense Q/K/V (G=4 groups of 128
features), full softmax attention within each (batch, group) pair, output
re-concatenated to [4, 4096, 512].

Sharding: B*G = 16 fully independent attention problems -> 2 per core.

v2 design (software-pipelined, fp16 compute):
  - prologue: load xg, cast fp16, PE-transpose to xgT [d, t];
    Q^T/K^T = W^T xgT (bias folded); V computed NATURAL directly via
    matmul(lhsT=xgT_chunk, rhs=Wv) with bv folded in (exact: softmax
    weights sum to 1, so out = P(V+bv)/den == PV/den + bv).
  - scores transposed: S^T[ts, tq] = K^T(lhsT) @ Q^T chunks; exp via ACT
    with 1/sqrt(gs) folded into the scale; probabilities pt land in fp16.
  - denominator: pt chunk pairs summed on DVE (fp16 2x mode), then 8
    one-column matmuls (lhsT=pair slice, rhs=ones) accumulate den^T[tq,1]
    directly in natural orientation -> tiny [128,8] PSUM tile per macro.
  - out^T accumulated over ts chunks in PSUM; epilogue per macro: DVE
    drains ps_out -> SBUF, approx-fast reciprocal of [128,8] den,
    PE-transpose 128-blocks to natural, per-partition tensor_scalar_mul
    by recip, DMA out.
  - emission is software-pipelined: out-matmuls lag 2 chunks behind
    scores/exp; the previous macro's epilogue is spread over slots in the
    first chunks of the next macro. PE never han an intentional idle slot
    (TRN2 PE drops to half clock after any idle gap).
"""

import os
import sys

sys.path.insert(0, "/opt/trn_rl_repo")

import numpy as np

import concourse.bass as bass
import concourse.mybir as mybir
import concourse.tile as tile
from concourse.masks import make_identity

B, T, F, G = 4, 4096, 512, 4
GS = F // G  # 128
N_CORES = 8
PAIRS_PER_CORE = (B * G) // N_CORES  # 2
TQ = 1024  # query tile width per macro
NM = T // TQ  # 4 macros
NCH = T // 128  # 32 key/time chunks
INV_SCALE = float(1.0 / (np.sqrt(np.float32(GS)) + 1e-9))

FP32 = mybir.dt.float32
FP16 = mybir.dt.float16

_NC_CACHE = None
_LAST_IN_MAPS = None


def _split_multi_waits(nc):
    """Walrus codegen rejects instructions carrying more than one semaphore
    wait on several instruction structs (DMA DIRECT2D, tensor_scalar, LDW).
    Hoist all-but-the-last wait of any multi-wait instruction onto same-engine
    NoOps inserted immediately before it: the sequencer executes them in
    order, so the gating semantics are identical."""
    n_split = 0
    for func in nc.m.functions:
        for block in func.blocks:
            new = []
            for inst in block.instructions:
                si = inst.sync_info
                waits = list(si.on_wait) if (si is not None and si.on_wait) else []
                if len(waits) > 1:
                    for w in waits[:-1]:
                        nop = mybir.InstNoOp(
                            name=nc.get_next_instruction_name(), ins=[], outs=[]
                        )
                        nop.engine = inst.engine
                        nop.sync_info = mybir.SyncInfo(on_wait=[w], on_update=[])
                        new.append(nop)
                        n_split += 1
                    inst.sync_info = mybir.SyncInfo(
                        on_wait=[waits[-1]],
                        on_update=list(si.on_update) if si.on_update else [],
                    )
                new.append(inst)
            block.instructions = new
    return n_split


def build_nc():
    nc = bass.Bass()

    ins = []
    outs = []
    outs_dbg = None
    if os.environ.get("DBG_DEN") == "1":
        outs_dbg = nc.declare_dram_parameter("dbg0", [128, 16], FP32, isOutput=True)
    for i in range(PAIRS_PER_CORE):
        ins.append(
            dict(
                x=nc.declare_dram_parameter(f"x{i}", [T, GS], FP32, isOutput=False),
                wq=nc.declare_dram_parameter(f"wq{i}", [GS, GS], FP32, isOutput=False),
                wk=nc.declare_dram_parameter(f"wk{i}", [GS, GS], FP32, isOutput=False),
                wv=nc.declare_dram_parameter(f"wv{i}", [GS, GS], FP32, isOutput=False),
                bq=nc.declare_dram_parameter(f"bq{i}", [1, GS], FP32, isOutput=False),
                bk=nc.declare_dram_parameter(f"bk{i}", [1, GS], FP32, isOutput=False),
                bv=nc.declare_dram_parameter(f"bv{i}", [1, GS], FP32, isOutput=False),
            )
        )
        outs.append(nc.declare_dram_parameter(f"y{i}", [T, GS], FP32, isOutput=True))

    with tile.TileContext(nc) as tc:
        with (
            tc.tile_pool(name="consts", bufs=1) as consts,
            tc.tile_pool(name="bigsb", bufs=2) as bigsb,  # per-pair persistent
            tc.tile_pool(name="ptp", bufs=4) as ptp,  # exp'd prob chunks
            tc.tile_pool(name="ppair", bufs=2) as ppair,  # chunk-pair sums
            tc.tile_pool(name="epi", bufs=2) as epi,  # epilogue sbuf tiles
            tc.tile_pool(name="ps", bufs=2, space="PSUM") as ps,  # scores + misc
            tc.tile_pool(name="ps_o", bufs=1, space="PSUM") as ps_o,  # out^T
            tc.tile_pool(name="ps_dn", bufs=1, space="PSUM") as ps_dn,  # den^T
            tc.tile_pool(name="ps_ep", bufs=1, space="PSUM") as ps_ep,  # nat out
        ):
            ident_h = consts.tile([128, 128], FP16)
            make_identity(nc, ident_h)
            ident_f = consts.tile([128, 128], FP32)
            make_identity(nc, ident_f)
            ones_col = consts.tile([128, 1], FP16)
            nc.vector.memset(ones_col, 1.0)

            # ------------- input DMAs for both pairs up front -------------
            # x arrives in 4 slab DMAs so casting/transposing can start as
            # soon as the first slab lands; weight DMAs precede bias DMAs so
            # the weight casts aren't gated behind the whole DMA queue.
            pair_in = []
            for i in range(PAIRS_PER_CORE):
                p = ins[i]
                xg_f = bigsb.tile([128, NCH, 128], FP32, tag="xg_f")
                for k in range(4):
                    nc.sync.dma_start(
                        out=xg_f[:, k * 8 : (k + 1) * 8, :],
                        in_=p["x"][k * 1024 : (k + 1) * 1024, :].rearrange(
                            "(c p) d -> p c d", p=128
                        ),
                    )
                pair_in.append(dict(xg_f=xg_f))
            for i in range(PAIRS_PER_CORE):
                p = ins[i]
                w_f = {}
                for nm in ("wq", "wk", "wv"):
                    wf = consts.tile([128, 128], FP32, tag=f"wf_{nm}{i}")
                    nc.gpsimd.dma_start(out=wf, in_=p[nm][:, :])
                    w_f[nm] = wf
                pair_in[i]["w_f"] = w_f
            for i in range(PAIRS_PER_CORE):
                p = ins[i]
                b_col = {}
                for nm in ("bq", "bk"):
                    bc = consts.tile([128, 1], FP32, tag=f"{nm}{i}")
                    nc.gpsimd.dma_start(
                        out=bc, in_=p[nm][:, :].rearrange("o d -> d o")
                    )
                    b_col[nm] = bc
                bvb = consts.tile([128, 128], FP32, tag=f"bvb{i}")
                _bv = p["bv"][:, :]
                nc.gpsimd.dma_start(
                    out=bvb,
                    in_=bass.AP(tensor=_bv.tensor, offset=_bv.offset,
                                ap=[[0, 128]] + list(_bv.ap[1:])),
                )
                pair_in[i]["b_col"] = b_col
                pair_in[i]["bvb"] = bvb

            # ------------- prologues (both pairs) -------------
            pair_data = []
            for i in range(PAIRS_PER_CORE):
                pi = pair_in[i]
                xg_h = bigsb.tile([128, NCH, 128], FP16, tag="xg_h")
                xgT = bigsb.tile([128, T], FP16, tag="xgT")
                # cast each slab, then transpose its 8 chunks in groups of 4
                # batched into one [128,512] psum tile -> single wide copy.
                for k in range(4):
                    nc.vector.tensor_copy(
                        xg_h[:, k * 8 : (k + 1) * 8, :],
                        pi["xg_f"][:, k * 8 : (k + 1) * 8, :],
                    )
                    for g in range(2):
                        c0 = k * 8 + g * 4
                        pst = ps.tile([128, 512], FP16, tag="sc")
                        for q in range(4):
                            nc.tensor.transpose(
                                pst[:, q * 128 : (q + 1) * 128],
                                xg_h[:, c0 + q, :], ident_h,
                            )
                        dsl = slice(c0 * 128, (c0 + 4) * 128)
                        if g % 2 == 0:
                            nc.vector.tensor_copy(xgT[:, dsl], pst)
                        else:
                            nc.scalar.copy(xgT[:, dsl], pst)

                w_h = {}
                for nm in ("wq", "wk", "wv"):
                    wh = consts.tile([128, 128], FP16, tag=f"wh_{nm}{i}")
                    nc.vector.tensor_copy(wh, pi["w_f"][nm])
                    w_h[nm] = wh

                # Q^T, K^T with bias
                qt = bigsb.tile([128, T], FP16, tag="qt")
                kt = bigsb.tile([128, T], FP16, tag="kt")
                for dst, wname, bname in ((qt, "wq", "bq"), (kt, "wk", "bk")):
                    for j in range(NM):
                        psq = ps.tile([128, TQ], FP32, tag="sc")
                        for h in range(2):
                            sl = slice(h * 512, (h + 1) * 512)
                            tsl = slice(j * TQ + h * 512, j * TQ + (h + 1) * 512)
                            nc.tensor.matmul(
                                psq[:, sl], w_h[wname], xgT[:, tsl],
                                start=True, stop=True,
                            )
                        dsl = slice(j * TQ, (j + 1) * TQ)
                        nc.vector.tensor_scalar_add(
                            dst[:, dsl], psq, pi["b_col"][bname]
                        )

                # V natural directly: v_nat[ts, e] = xgT_chunk^T @ Wv
                # (bv is folded into the epilogue normalize: out = PV/den + bv)
                # 4 chunk-matmuls batched per [128,512] psum tile, one drain.
                v_nat = bigsb.tile([128, T], FP16, tag="v_nat")
                for g in range(NCH // 4):
                    c0 = g * 4
                    psv = ps.tile([128, 512], FP32, tag="sc")
                    for q in range(4):
                        c = c0 + q
                        nc.tensor.matmul(
                            psv[:, q * 128 : (q + 1) * 128],
                            xgT[:, c * 128 : (c + 1) * 128], w_h["wv"],
                            start=True, stop=True,
                        )
                    dsl = slice(c0 * 128, (c0 + 4) * 128)
                    if g % 2 == 0:
                        nc.vector.tensor_copy(v_nat[:, dsl], psv)
                    else:
                        nc.scalar.copy(v_nat[:, dsl], psv)

                pair_data.append(dict(qt=qt, kt=kt, v_nat=v_nat, bvb=pi["bvb"]))

            # ------------- attention: software-pipelined macro loop -------
            # prev: state of the previous macro whose epilogue is pending.
            prev = None

            def emit_epilogue_slot(c, st):
                """Emit the slice of the previous macro's epilogue assigned
                to chunk-slot c of the current macro stream."""
                if st is None:
                    return
                if c == 1:
                    # drain ps_out -> SBUF (DVE), reciprocal of den
                    nc.vector.tensor_copy(st["osb"], st["ps_out"])
                    nc.vector.reciprocal_approx_fast(
                        out=st["rcol"], in_=st["ps_den"]
                    )
                    if st.get("dbg") is not None:
                        dbg_sb = consts.tile([128, 16], FP32, tag="dbg_sb")
                        nc.vector.tensor_copy(dbg_sb[:, 0:8], st["ps_den"])
                        nc.vector.tensor_copy(dbg_sb[:, 8:16], st["rcol"])
                        st["dbg_sb"] = dbg_sb
                elif c in (3, 4):
                    j0 = 0 if c == 3 else 4
                    for j in range(j0, j0 + 4):
                        q = j % 4
                        pse = st["ps_ep_t"]
                        nc.tensor.transpose(
                            pse[:, q * 128 : (q + 1) * 128],
                            st["osb"][:, j * 128 : (j + 1) * 128],
                            ident_f,
                        )
                        # out = (outT.T * 1/den) + bv  (bvb: partition-
                        # constant broadcast of bv, varies along e = free)
                        nc.vector.scalar_tensor_tensor(
                            out=st["out_sb"][:, j, :],
                            in0=pse[:, q * 128 : (q + 1) * 128],
                            scalar=st["rcol"][:, j : j + 1],
                            in1=st["bvb"],
                            op0=mybir.AluOpType.mult,
                            op1=mybir.AluOpType.add,
                        )
                elif c == 5:
                    nc.sync.dma_start(
                        out=st["y"][st["tq0"] : st["tq0"] + TQ, :].rearrange(
                            "(c p) d -> p c d", p=128
                        ),
                        in_=st["out_sb"],
                    )


            def emit_flush(st_cur, ptl, ppl):
                """Final out/den matmuls of the current macro (lag drain)."""
                for cc in (NCH - 2, NCH - 1):
                    for h in range(2):
                        sl = slice(h * 512, (h + 1) * 512)
                        nc.tensor.matmul(
                            st_cur["ps_out"][:, sl],
                            st_cur["v_nat"][:, cc * 128 : (cc + 1) * 128],
                            ptl[cc][:, sl],
                            start=(cc == 0), stop=(cc == NCH - 1),
                        )
                # NOTE: stop=True only on the final matmul touching the bank.
                # A matmul with start=False, stop=True whose write is NOT the
                # bank-final one loses its accumulation on TRN2 hardware
                # (observed: den columns 0..6 missing the last pair's sum).
                pp = NCH // 2 - 1  # last pair
                for j in range(8):
                    nc.tensor.matmul(
                        st_cur["ps_den"][:, j : j + 1],
                        ppl[pp][:, j * 128 : (j + 1) * 128],
                        ones_col,
                        start=(pp == 0), stop=(j == 7),
                    )

            for i in range(PAIRS_PER_CORE):
                pd = pair_data[i]
                qt, kt, v_nat = pd["qt"], pd["kt"], pd["v_nat"]
                for m in range(NM):
                    tq0 = m * TQ
                    ps_out = ps_o.tile([128, TQ], FP32, tag="o")
                    ps_den = ps_dn.tile([128, 8], FP32, tag="dn")
                    ptl = [None] * NCH
                    ppl = [None] * (NCH // 2)
                    st_cur = dict(
                        ps_out=ps_out, ps_den=ps_den, v_nat=v_nat, tq0=tq0,
                        y=outs[i], bvb=pd["bvb"],
                        dbg=outs_dbg if (i == 1 and m == NM - 1) else None,
                    )
                    for c in range(NCH):
                        # scores for chunk c
                        sc_t = ps.tile([128, TQ], FP32, tag="sc")
                        ksl = kt[:, c * 128 : (c + 1) * 128]
                        for h in range(2):
                            sl = slice(h * 512, (h + 1) * 512)
                            qsl = slice(tq0 + h * 512, tq0 + (h + 1) * 512)
                            nc.tensor.matmul(
                                sc_t[:, sl], ksl, qt[:, qsl], start=True, stop=True
                            )
                        pt_c = ptp.tile([128, TQ], FP16, tag="pt")
                        nc.scalar.activation(
                            pt_c, sc_t, mybir.ActivationFunctionType.Exp,
                            scale=INV_SCALE,
                        )
                        ptl[c] = pt_c

                        # previous macro's epilogue rides in early slots
                        emit_epilogue_slot(c, prev)

                        # lagged out-matmuls (chunk c-2)
                        if c >= 2:
                            cc = c - 2
                            for h in range(2):
                                sl = slice(h * 512, (h + 1) * 512)
                                nc.tensor.matmul(
                                    ps_out[:, sl], v_nat[:, cc * 128 : (cc + 1) * 128], ptl[cc][:, sl],
                                    start=(cc == 0), stop=False,
                                )

                        # chunk-pair sum; lagged den matmuls
                        if c % 2 == 1:
                            pp_i = (c - 1) // 2
                            pp_t = ppair.tile([128, TQ], FP16, tag="pp")
                            nc.vector.tensor_add(pp_t, ptl[c - 1], ptl[c])
                            ppl[pp_i] = pp_t
                            if c >= 3:
                                dp = (c - 3) // 2
                                for j in range(8):
                                    # start=True ONLY on the very first
                                    # column-matmul: a start marks the whole
                                    # 2KB psum zero-region pending-zero, so
                                    # start on later columns would discard
                                    # the columns written just before.
                                    nc.tensor.matmul(
                                        ps_den[:, j : j + 1],
                                        ppl[dp][:, j * 128 : (j + 1) * 128],
                                        ones_col,
                                        start=(dp == 0 and j == 0), stop=False,
                                    )
                    # flush the lag of this macro
                    emit_flush(st_cur, ptl, ppl)
                    # hand off epilogue state
                    osb = epi.tile([128, TQ], FP32, tag="osb")
                    rcol = epi.tile([128, 8], FP32, tag="rcol")
                    out_sb = epi.tile([128, TQ // 128, 128], FP32, tag="out_sb")
                    ps_ep_t = ps_ep.tile([128, 512], FP32, tag="ep")
                    st_cur.update(osb=osb, rcol=rcol, out_sb=out_sb, ps_ep_t=ps_ep_t)
                    prev = st_cur

            # tail: last macro's epilogue, emitted standalone
            for c in (1, 3, 4, 5):
                emit_epilogue_slot(c, prev)
            if prev.get("dbg") is not None:
                # late re-read of the final macro's den psum, after everything
                nc.vector.tensor_copy(prev["dbg_sb"][:, 8:16], prev["ps_den"])
                nc.sync.dma_start(out=prev["dbg"][:, :], in_=prev["dbg_sb"])

    # populate .instr bytes for extended InstISA ops (custom DVE reciprocal);
    # raw Bass skips this pass and walrus then fails with "ISA wrong length".
    mybir.codegen_inst_isa_subclasses(nc)
    _split_multi_waits(nc)
    return nc


def _get_nc():
    global _NC_CACHE
    if _NC_CACHE is None:
        _NC_CACHE = build_nc()
    return _NC_CACHE


def kernel(**inputs: np.ndarray) -> np.ndarray:
    x = np.ascontiguousarray(inputs["x"], dtype=np.float32)
    Wq = np.asarray(inputs["Wq"], dtype=np.float32)
    Wk = np.asarray(inputs["Wk"], dtype=np.float32)
    Wv = np.asarray(inputs["Wv"], dtype=np.float32)
    bq = np.asarray(inputs["bq"], dtype=np.float32)
    bk = np.asarray(inputs["bk"], dtype=np.float32)
    bv = np.asarray(inputs["bv"], dtype=np.float32)

    nc = _get_nc()

    in_maps = []
    for core in range(N_CORES):
        m = {}
        for i in range(PAIRS_PER_CORE):
            pair = core * PAIRS_PER_CORE + i
            b, g = pair // G, pair % G
            sl = slice(g * GS, (g + 1) * GS)
            m[f"x{i}"] = np.ascontiguousarray(x[b, :, sl])
            m[f"wq{i}"] = np.ascontiguousarray(Wq[g])
            m[f"wk{i}"] = np.ascontiguousarray(Wk[g])
            m[f"wv{i}"] = np.ascontiguousarray(Wv[g])
            m[f"bq{i}"] = np.ascontiguousarray(bq[g].reshape(1, GS))
            m[f"bk{i}"] = np.ascontiguousarray(bk[g].reshape(1, GS))
            m[f"bv{i}"] = np.ascontiguousarray(bv[g].reshape(1, GS))
        in_maps.append(m)

    global _LAST_IN_MAPS
    _LAST_IN_MAPS = in_maps

    from concourse.bass_utils import run_bass_kernel_spmd

    res = run_bass_kernel_spmd(nc, in_maps, list(range(N_CORES)))

    y = np.empty((B, T, F), dtype=np.float32)
    for core in range(N_CORES):
        for i in range(PAIRS_PER_CORE):
            pair = core * PAIRS_PER_CORE + i
            b, g = pair // G, pair % G
            y[b, :, g * GS : (g + 1) * GS] = res.results[core][f"y{i}"]
    return y


# revision 23
# speedup vs baseline: 1.9578x; 1.0086x over previous
"""Grouped-Query Attention kernel for Trainium2 (8 NeuronCores, SPMD).

Problem: x [4, 4096, 512] fp32, per-group Dense Q/K/V (G=4 groups of 128
features), full softmax attention within each (batch, group) pair, output
re-concatenated to [4, 4096, 512].

Sharding: B*G = 16 fully independent attention problems -> 2 per core.

v2 design (software-pipelined, fp16 compute):
  - prologue: load xg, cast fp16, PE-transpose to xgT [d, t];
    Q^T/K^T = W^T xgT (bias folded); V computed NATURAL directly via
    matmul(lhsT=xgT_chunk, rhs=Wv) with bv folded in (exact: softmax
    weights sum to 1, so out = P(V+bv)/den == PV/den + bv).
  - scores transposed: S^T[ts, tq] = K^T(lhsT) @ Q^T chunks; exp via ACT
    with 1/sqrt(gs) folded into the scale; probabilities pt land in fp16.
  - denominator: pt chunk pairs summed on DVE (fp16 2x mode), then 8
    one-column matmuls (lhsT=pair slice, rhs=ones) accumulate den^T[tq,1]
    directly in natural orientation -> tiny [128,8] PSUM tile per macro.
  - out^T accumulated over ts chunks in PSUM; epilogue per macro: DVE
    drains ps_out -> SBUF, approx-fast reciprocal of [128,8] den,
    PE-transpose 128-blocks to natural, per-partition tensor_scalar_mul
    by recip, DMA out.
  - emission is software-pipelined: out-matmuls lag 2 chunks behind
    scores/exp; the previous macro's epilogue is spread over slots in the
    first chunks of the next macro. PE never han an intentional idle slot
    (TRN2 PE drops to half clock after any idle gap).
"""

import os
import sys

sys.path.insert(0, "/opt/trn_rl_repo")

import numpy as np

import concourse.bass as bass
import concourse.mybir as mybir
import concourse.tile as tile
from concourse.masks import make_identity

B, T, F, G = 4, 4096, 512, 4
GS = F // G  # 128
N_CORES = 8
PAIRS_PER_CORE = (B * G) // N_CORES  # 2
TQ = 1024  # query tile width per macro
NM = T // TQ  # 4 macros
NCH = T // 128  # 32 key/time chunks
INV_SCALE = float(1.0 / (np.sqrt(np.float32(GS)) + 1e-9))

FP32 = mybir.dt.float32
FP16 = mybir.dt.float16

_NC_CACHE = None
_LAST_IN_MAPS = None


def _split_multi_waits(nc):
    """Walrus codegen rejects instructions carrying more than one semaphore
    wait on several instruction structs (DMA DIRECT2D, tensor_scalar, LDW).
    Hoist all-but-the-last wait of any multi-wait instruction onto same-engine
    NoOps inserted immediately before it: the sequencer executes them in
    order, so the gating semantics are identical."""
    n_split = 0
    for func in nc.m.functions:
        for block in func.blocks:
            new = []
            for inst in block.instructions:
                si = inst.sync_info
                waits = list(si.on_wait) if (si is not None and si.on_wait) else []
                if len(waits) > 1:
                    for w in waits[:-1]:
                        nop = mybir.InstNoOp(
                            name=nc.get_next_instruction_name(), ins=[], outs=[]
                        )
                        nop.engine = inst.engine
                        nop.sync_info = mybir.SyncInfo(on_wait=[w], on_update=[])
                        new.append(nop)
                        n_split += 1
                    inst.sync_info = mybir.SyncInfo(
                        on_wait=[waits[-1]],
                        on_update=list(si.on_update) if si.on_update else [],
                    )
                new.append(inst)
            block.instructions = new
    return n_split


def build_nc():
    nc = bass.Bass()

    ins = []
    outs = []
    outs_dbg = None
    if os.environ.get("DBG_DEN") == "1":
        outs_dbg = nc.declare_dram_parameter("dbg0", [128, 16], FP32, isOutput=True)
    for i in range(PAIRS_PER_CORE):
        ins.append(
            dict(
                x=nc.declare_dram_parameter(f"x{i}", [T, GS], FP32, isOutput=False),
                wq=nc.declare_dram_parameter(f"wq{i}", [GS, GS], FP32, isOutput=False),
                wk=nc.declare_dram_parameter(f"wk{i}", [GS, GS], FP32, isOutput=False),
                wv=nc.declare_dram_parameter(f"wv{i}", [GS, GS], FP32, isOutput=False),
                bq=nc.declare_dram_parameter(f"bq{i}", [1, GS], FP32, isOutput=False),
                bk=nc.declare_dram_parameter(f"bk{i}", [1, GS], FP32, isOutput=False),
                bv=nc.declare_dram_parameter(f"bv{i}", [1, GS], FP32, isOutput=False),
            )
        )
        outs.append(nc.declare_dram_parameter(f"y{i}", [T, GS], FP32, isOutput=True))

    with tile.TileContext(nc) as tc:
        with (
            tc.tile_pool(name="consts", bufs=1) as consts,
            tc.tile_pool(name="bigsb", bufs=2) as bigsb,  # per-pair persistent
            tc.tile_pool(name="ptp", bufs=4) as ptp,  # exp'd prob chunks
            tc.tile_pool(name="ppair", bufs=2) as ppair,  # chunk-pair sums
            tc.tile_pool(name="epi", bufs=2) as epi,  # epilogue sbuf tiles
            tc.tile_pool(name="ps", bufs=2, space="PSUM") as ps,  # scores + misc
            tc.tile_pool(name="ps_o", bufs=1, space="PSUM") as ps_o,  # out^T
            tc.tile_pool(name="ps_dn", bufs=1, space="PSUM") as ps_dn,  # den^T
            tc.tile_pool(name="ps_ep", bufs=1, space="PSUM") as ps_ep,  # nat out
        ):
            ident_h = consts.tile([128, 128], FP16)
            make_identity(nc, ident_h)
            ident_f = consts.tile([128, 128], FP32)
            make_identity(nc, ident_f)
            ones_col = consts.tile([128, 1], FP16)
            nc.vector.memset(ones_col, 1.0)

            # ------------- input DMAs for both pairs up front -------------
            # x arrives in 4 slab DMAs so casting/transposing can start as
            # soon as the first slab lands; weight DMAs precede bias DMAs so
            # the weight casts aren't gated behind the whole DMA queue.
            pair_in = []
            for i in range(PAIRS_PER_CORE):
                p = ins[i]
                xg_f = bigsb.tile([128, NCH, 128], FP32, tag="xg_f")
                for k in range(4):
                    nc.sync.dma_start(
                        out=xg_f[:, k * 8 : (k + 1) * 8, :],
                        in_=p["x"][k * 1024 : (k + 1) * 1024, :].rearrange(
                            "(c p) d -> p c d", p=128
                        ),
                    )
                pair_in.append(dict(xg_f=xg_f))
            for i in range(PAIRS_PER_CORE):
                p = ins[i]
                w_f = {}
                for nm in ("wq", "wk", "wv"):
                    wf = consts.tile([128, 128], FP32, tag=f"wf_{nm}{i}")
                    nc.gpsimd.dma_start(out=wf, in_=p[nm][:, :])
                    w_f[nm] = wf
                pair_in[i]["w_f"] = w_f
            for i in range(PAIRS_PER_CORE):
                p = ins[i]
                b_col = {}
                for nm in ("bq", "bk"):
                    bc = consts.tile([128, 1], FP32, tag=f"{nm}{i}")
                    nc.gpsimd.dma_start(
                        out=bc, in_=p[nm][:, :].rearrange("o d -> d o")
                    )
                    b_col[nm] = bc
                bvb = consts.tile([128, 128], FP32, tag=f"bvb{i}")
                _bv = p["bv"][:, :]
                nc.gpsimd.dma_start(
                    out=bvb,
                    in_=bass.AP(tensor=_bv.tensor, offset=_bv.offset,
                                ap=[[0, 128]] + list(_bv.ap[1:])),
                )
                pair_in[i]["b_col"] = b_col
                pair_in[i]["bvb"] = bvb

            # ------------- prologues (both pairs) -------------
            pair_data = []
            for i in range(PAIRS_PER_CORE):
                pi = pair_in[i]
                xg_h = bigsb.tile([128, NCH, 128], FP16, tag="xg_h")
                xgT = bigsb.tile([128, T], FP16, tag="xgT")
                # cast each slab, then transpose its 8 chunks in groups of 4
                # batched into one [128,512] psum tile -> single wide copy.
                for k in range(4):
                    nc.vector.tensor_copy(
                        xg_h[:, k * 8 : (k + 1) * 8, :],
                        pi["xg_f"][:, k * 8 : (k + 1) * 8, :],
                    )
                    for g in range(2):
                        c0 = k * 8 + g * 4
                        pst = ps.tile([128, 512], FP16, tag="sc")
                        for q in range(4):
                            nc.tensor.transpose(
                                pst[:, q * 128 : (q + 1) * 128],
                                xg_h[:, c0 + q, :], ident_h,
                            )
                        dsl = slice(c0 * 128, (c0 + 4) * 128)
                        if g % 2 == 0:
                            nc.vector.tensor_copy(xgT[:, dsl], pst)
                        else:
                            nc.scalar.copy(xgT[:, dsl], pst)

                w_h = {}
                for nm in ("wq", "wk", "wv"):
                    wh = consts.tile([128, 128], FP16, tag=f"wh_{nm}{i}")
                    nc.vector.tensor_copy(wh, pi["w_f"][nm])
                    w_h[nm] = wh

                # Q^T, K^T with bias
                qt = bigsb.tile([128, T], FP16, tag="qt")
                kt = bigsb.tile([128, T], FP16, tag="kt")
                for dst, wname, bname in ((qt, "wq", "bq"), (kt, "wk", "bk")):
                    for j in range(NM):
                        psq = ps.tile([128, TQ], FP32, tag="sc")
                        for h in range(2):
                            sl = slice(h * 512, (h + 1) * 512)
                            tsl = slice(j * TQ + h * 512, j * TQ + (h + 1) * 512)
                            nc.tensor.matmul(
                                psq[:, sl], w_h[wname], xgT[:, tsl],
                                start=True, stop=True,
                            )
                        dsl = slice(j * TQ, (j + 1) * TQ)
                        if j % 2 == 0:
                            nc.vector.tensor_scalar_add(
                                dst[:, dsl], psq, pi["b_col"][bname]
                            )
                        else:
                            nc.scalar.add(dst[:, dsl], psq, pi["b_col"][bname])

                # V natural directly: v_nat[ts, e] = xgT_chunk^T @ Wv
                # (bv is folded into the epilogue normalize: out = PV/den + bv)
                # 4 chunk-matmuls batched per [128,512] psum tile, one drain.
                v_nat = bigsb.tile([128, T], FP16, tag="v_nat")
                for g in range(NCH // 4):
                    c0 = g * 4
                    psv = ps.tile([128, 512], FP32, tag="sc")
                    for q in range(4):
                        c = c0 + q
                        nc.tensor.matmul(
                            psv[:, q * 128 : (q + 1) * 128],
                            xgT[:, c * 128 : (c + 1) * 128], w_h["wv"],
                            start=True, stop=True,
                        )
                    dsl = slice(c0 * 128, (c0 + 4) * 128)
                    if g % 2 == 0:
                        nc.vector.tensor_copy(v_nat[:, dsl], psv)
                    else:
                        nc.scalar.copy(v_nat[:, dsl], psv)

                pair_data.append(dict(qt=qt, kt=kt, v_nat=v_nat, bvb=pi["bvb"]))

            # ------------- attention: software-pipelined macro loop -------
            # prev: state of the previous macro whose epilogue is pending.
            prev = None

            def emit_epilogue_slot(c, st):
                """Emit the slice of the previous macro's epilogue assigned
                to chunk-slot c of the current macro stream."""
                if st is None:
                    return
                if c == 1:
                    # drain ps_out -> SBUF (DVE), reciprocal of den
                    nc.vector.tensor_copy(st["osb"], st["ps_out"])
                    nc.vector.reciprocal_approx_fast(
                        out=st["rcol"], in_=st["ps_den"]
                    )
                    if st.get("dbg") is not None:
                        dbg_sb = consts.tile([128, 16], FP32, tag="dbg_sb")
                        nc.vector.tensor_copy(dbg_sb[:, 0:8], st["ps_den"])
                        nc.vector.tensor_copy(dbg_sb[:, 8:16], st["rcol"])
                        st["dbg_sb"] = dbg_sb
                elif c in (3, 4):
                    j0 = 0 if c == 3 else 4
                    for j in range(j0, j0 + 4):
                        q = j if st.get("ep_wide") else j % 4
                        pse = st["ps_ep_t"]
                        nc.tensor.transpose(
                            pse[:, q * 128 : (q + 1) * 128],
                            st["osb"][:, j * 128 : (j + 1) * 128],
                            ident_h,
                        )
                        # out = (outT.T * 1/den) + bv  (bvb: partition-
                        # constant broadcast of bv, varies along e = free)
                        nc.vector.scalar_tensor_tensor(
                            out=st["out_sb"][:, j, :],
                            in0=pse[:, q * 128 : (q + 1) * 128],
                            scalar=st["rcol"][:, j : j + 1],
                            in1=st["bvb"],
                            op0=mybir.AluOpType.mult,
                            op1=mybir.AluOpType.add,
                        )
                elif c == 5:
                    nc.sync.dma_start(
                        out=st["y"][st["tq0"] : st["tq0"] + TQ, :].rearrange(
                            "(c p) d -> p c d", p=128
                        ),
                        in_=st["out_sb"],
                    )


            def emit_flush(st_cur, ptl, ppl):
                """Final out/den matmuls of the current macro (lag drain)."""
                for cc in (NCH - 2, NCH - 1):
                    for h in range(2):
                        sl = slice(h * 512, (h + 1) * 512)
                        nc.tensor.matmul(
                            st_cur["ps_out"][:, sl],
                            st_cur["v_nat"][:, cc * 128 : (cc + 1) * 128],
                            ptl[cc][:, sl],
                            start=(cc == 0), stop=(cc == NCH - 1),
                        )
                # NOTE: stop=True only on the final matmul touching the bank.
                # A matmul with start=False, stop=True whose write is NOT the
                # bank-final one loses its accumulation on TRN2 hardware
                # (observed: den columns 0..6 missing the last pair's sum).
                pp = NCH // 2 - 1  # last pair
                for j in range(8):
                    nc.tensor.matmul(
                        st_cur["ps_den"][:, j : j + 1],
                        ppl[pp][:, j * 128 : (j + 1) * 128],
                        ones_col,
                        start=(pp == 0), stop=(j == 7),
                    )

            for i in range(PAIRS_PER_CORE):
                pd = pair_data[i]
                qt, kt, v_nat = pd["qt"], pd["kt"], pd["v_nat"]
                for m in range(NM):
                    tq0 = m * TQ
                    ps_out = ps_o.tile([128, TQ], FP32, tag="o")
                    ps_den = ps_dn.tile([128, 8], FP32, tag="dn")
                    ptl = [None] * NCH
                    ppl = [None] * (NCH // 2)
                    st_cur = dict(
                        ps_out=ps_out, ps_den=ps_den, v_nat=v_nat, tq0=tq0,
                        y=outs[i], bvb=pd["bvb"],
                        dbg=outs_dbg if (i == 1 and m == NM - 1) else None,
                    )
                    for c in range(NCH):
                        # scores for chunk c
                        sc_t = ps.tile([128, TQ], FP32, tag="sc")
                        ksl = kt[:, c * 128 : (c + 1) * 128]
                        for h in range(2):
                            sl = slice(h * 512, (h + 1) * 512)
                            qsl = slice(tq0 + h * 512, tq0 + (h + 1) * 512)
                            nc.tensor.matmul(
                                sc_t[:, sl], ksl, qt[:, qsl], start=True, stop=True
                            )
                        pt_c = ptp.tile([128, TQ], FP16, tag="pt")
                        nc.scalar.activation(
                            pt_c, sc_t, mybir.ActivationFunctionType.Exp,
                            scale=INV_SCALE,
                        )
                        ptl[c] = pt_c

                        # previous macro's epilogue rides in early slots
                        emit_epilogue_slot(c, prev)

                        # lagged out-matmuls (chunk c-2)
                        if c >= 2:
                            cc = c - 2
                            for h in range(2):
                                sl = slice(h * 512, (h + 1) * 512)
                                nc.tensor.matmul(
                                    ps_out[:, sl], v_nat[:, cc * 128 : (cc + 1) * 128], ptl[cc][:, sl],
                                    start=(cc == 0), stop=False,
                                )

                        # chunk-pair sum; lagged den matmuls
                        if c % 2 == 1:
                            pp_i = (c - 1) // 2
                            pp_t = ppair.tile([128, TQ], FP16, tag="pp")
                            nc.vector.tensor_add(pp_t, ptl[c - 1], ptl[c])
                            ppl[pp_i] = pp_t
                            if c >= 3:
                                dp = (c - 3) // 2
                                for j in range(8):
                                    # start=True ONLY on the very first
                                    # column-matmul: a start marks the whole
                                    # 2KB psum zero-region pending-zero, so
                                    # start on later columns would discard
                                    # the columns written just before.
                                    nc.tensor.matmul(
                                        ps_den[:, j : j + 1],
                                        ppl[dp][:, j * 128 : (j + 1) * 128],
                                        ones_col,
                                        start=(dp == 0 and j == 0), stop=False,
                                    )
                    # flush the lag of this macro
                    emit_flush(st_cur, ptl, ppl)
                    # hand off epilogue state
                    osb = epi.tile([128, TQ], FP16, tag="osb")
                    rcol = epi.tile([128, 8], FP32, tag="rcol")
                    out_sb = epi.tile([128, TQ // 128, 128], FP32, tag="out_sb")
                    ps_ep_t = ps_ep.tile([128, 512], FP16, tag="ep")
                    st_cur.update(osb=osb, rcol=rcol, out_sb=out_sb, ps_ep_t=ps_ep_t)
                    prev = st_cur

            # tail: last macro's epilogue, emitted standalone; the sc
            # pool banks are free now, so use a wide psum tile (8 blocks,
            # no WAR ping-pong between transpose halves).
            prev["ps_ep_t"] = ps.tile([128, TQ], FP16, tag="sc", name="ep_tail")
            prev["ep_wide"] = True
            for c in (1, 3, 4, 5):
                emit_epilogue_slot(c, prev)
            if prev.get("dbg") is not None:
                # late re-read of the final macro's den psum, after everything
                nc.vector.tensor_copy(prev["dbg_sb"][:, 8:16], prev["ps_den"])
                nc.sync.dma_start(out=prev["dbg"][:, :], in_=prev["dbg_sb"])

    # populate .instr bytes for extended InstISA ops (custom DVE reciprocal);
    # raw Bass skips this pass and walrus then fails with "ISA wrong length".
    mybir.codegen_inst_isa_subclasses(nc)
    _split_multi_waits(nc)
    return nc


def _get_nc():
    global _NC_CACHE
    if _NC_CACHE is None:
        _NC_CACHE = build_nc()
    return _NC_CACHE


def kernel(**inputs: np.ndarray) -> np.ndarray:
    x = np.ascontiguousarray(inputs["x"], dtype=np.float32)
    Wq = np.asarray(inputs["Wq"], dtype=np.float32)
    Wk = np.asarray(inputs["Wk"], dtype=np.float32)
    Wv = np.asarray(inputs["Wv"], dtype=np.float32)
    bq = np.asarray(inputs["bq"], dtype=np.float32)
    bk = np.asarray(inputs["bk"], dtype=np.float32)
    bv = np.asarray(inputs["bv"], dtype=np.float32)

    nc = _get_nc()

    in_maps = []
    for core in range(N_CORES):
        m = {}
        for i in range(PAIRS_PER_CORE):
            pair = core * PAIRS_PER_CORE + i
            b, g = pair // G, pair % G
            sl = slice(g * GS, (g + 1) * GS)
            m[f"x{i}"] = np.ascontiguousarray(x[b, :, sl])
            m[f"wq{i}"] = np.ascontiguousarray(Wq[g])
            m[f"wk{i}"] = np.ascontiguousarray(Wk[g])
            m[f"wv{i}"] = np.ascontiguousarray(Wv[g])
            m[f"bq{i}"] = np.ascontiguousarray(bq[g].reshape(1, GS))
            m[f"bk{i}"] = np.ascontiguousarray(bk[g].reshape(1, GS))
            m[f"bv{i}"] = np.ascontiguousarray(bv[g].reshape(1, GS))
        in_maps.append(m)

    global _LAST_IN_MAPS
    _LAST_IN_MAPS = in_maps

    from concourse.bass_utils import run_bass_kernel_spmd

    res = run_bass_kernel_spmd(nc, in_maps, list(range(N_CORES)))

    y = np.empty((B, T, F), dtype=np.float32)
    for core in range(N_CORES):
        for i in range(PAIRS_PER_CORE):
            pair = core * PAIRS_PER_CORE + i
            b, g = pair // G, pair % G
            y[b, :, g * GS : (g + 1) * GS] = res.results[core][f"y{i}"]
    return y
